# revision 49
# baseline (speedup 1.0000x reference)
"""Trainium2 Bass kernel for nn_Attention_66563403153646.

Dense transformer attention block with rotary embeddings + gated adapter
(prefix) attention, fp32 reference:

    y = softmax(rope(x@wq) @ rope(x@wk).T * k + mask) @ (x@wv)
      + gate * softmax(rope(x@wq) @ (adapter@wk).T * k) @ (adapter@wv)
    out = y @ wo

Sharding: 4-way tensor-parallel over heads x 2-way data-parallel over batch
(8 NeuronCores). Each core computes a [S, D] partial of its batch's output
(its 8 heads' contribution through wo); the host sums the 4 TP partials
(that reduction is free host-side, and the o@wo work per core is invariant
to any other split of the contraction across the TP group).

All matmul operands are bf16 (PE 1 cycle/row); accumulation is fp32 in
PSUM. The kernel is PE-cycle-bound, so the schedule is a fused per-HEAD
pipeline: project q/k/v for head h (x streamed in 512-col tiles, the three
weight slices resident), rope in place, then run head h's attention while
head h+1's inputs prefetch -- the per-head softmax exp burst on the ACT
engine hides under the next head's projection matmuls, and q/k/v only ever
need per-head SBUF tiles. Layouts:
  - x is fed transposed ([D, S]) so projections contract D on partitions.
  - q/k live per-head in [HD, S] layout with rope-pair-permuted head dims
    (host permutes wq/wk columns: even rope dims first, odd second) so
    RoPE is two partition-halves of elementwise ops.
  - v is transposed to natural [s, d] per 512-col strip via DMA-XBAR.
  - scores are computed per 512-col q tile over 128-row k blocks
    (causal: only blocks <= diagonal; the diagonal band gets a 0/1
    triangle mask on the Pool engine).
  - p blocks land in [k, q] layout, which is exactly what p@v needs; the
    normalized o evicts to DRAM scratch per (head, q-tile) and streams
    back for the wo pass in 512-row groups.
  - adapter projections (ak/av, gate folded into av) are precomputed on
    the host (tiny: 10 x D x 2DH); the per-q-block adapter score matmuls
    ride inside the projection pass.
"""

import sys

sys.path.insert(0, "/opt/trn_rl_repo")

import math
from contextlib import ExitStack
from dataclasses import dataclass

import numpy as np

import concourse.bass as bass
import concourse.mybir as mybir
import concourse.tile as tile
from concourse import bacc
from concourse.masks import make_identity

f32 = mybir.dt.float32
f32r = mybir.dt.float32r
bf16 = mybir.dt.bfloat16

P = 128


@dataclass(frozen=True)
class Cfg:
    S: int = 2048  # sequence length
    D: int = 4096  # model dim
    HPC: int = 8  # heads per core
    HD: int = 128  # head dim
    AL: int = 10  # adapter len
    mm_dt: object = bf16  # matmul operand dtype
    pipe_depth: int = 2  # phase-2 software pipeline depth

    @property
    def DC(self):  # D chunks of 128 (contraction)
        return self.D // P

    @property
    def DH(self):  # head-slice width
        return self.HPC * self.HD

    @property
    def NQ(self):  # 128-row q blocks
        return self.S // P

    @property
    def NT(self):  # 512-col tiles
        return self.S // 512

    @property
    def XS(self):  # phase-1 x s-tile width
        return 512


def build_nc(cfg: Cfg, phases=(1, 2, 3)):
    nc = bacc.Bacc(None, target_bir_lowering=False, debug=False, num_devices=8)
    S, D, HPC, HD, AL = cfg.S, cfg.D, cfg.HPC, cfg.HD, cfg.AL
    DC, DH, NQ, NT, XS = cfg.DC, cfg.DH, cfg.NQ, cfg.NT, cfg.XS
    mdt = cfg.mm_dt
    inv_sqrt = 1.0 / math.sqrt(HD)
    HH = HD // 2
    WC3 = HPC  # phase-3 contraction chunks (own heads)
    ET3 = D // 512  # full-width output tiles; host sums the 4 TP partials

    # ---- I/O ----
    # All big operands arrive host-pre-arranged in partition-major layouts
    # so every DMA is one large contiguous line per partition.
    xT_d = nc.dram_tensor("xT", [P, DC, S], mdt, kind="ExternalInput")
    wq_d = nc.dram_tensor("wq", [P, HPC, DC, HD], mdt, kind="ExternalInput")
    wk_d = nc.dram_tensor("wk", [P, HPC, DC, HD], mdt, kind="ExternalInput")
    wv_d = nc.dram_tensor("wv", [P, HPC, DC, HD], mdt, kind="ExternalInput")
    wo_d = nc.dram_tensor("wo", [P, ET3, WC3, 512], mdt, kind="ExternalInput")
    # host-computed adapter projections (tiny): akT in rope-permuted basis,
    # av with the per-head gate folded in
    akT_d = nc.dram_tensor("akT", [P, HPC, AL], mdt, kind="ExternalInput")
    av_d = nc.dram_tensor("av", [AL, HPC, P], mdt, kind="ExternalInput")
    cosT_d = nc.dram_tensor("cosT", [HH, S], f32, kind="ExternalInput")
    sinT_d = nc.dram_tensor("sinT", [HH, S], f32, kind="ExternalInput")
    # 0/1 upper-triangular (incl diag) [k,q] mask for the boundary block
    tri_d = nc.dram_tensor("tri", [P, P], f32, kind="ExternalInput")
    y_d = nc.dram_tensor("y", [S, D], f32, kind="ExternalOutput")

    ExpF = mybir.ActivationFunctionType.Exp
    AX = mybir.AxisListType.X
    Mul = mybir.AluOpType.mult

    with tile.TileContext(nc) as tc:
        with (
            tc.tile_pool(name="persist", bufs=1) as persist,
            tc.tile_pool(name="ccdram", bufs=1, space="DRAM") as ccdram,
        ):
            # DRAM scratch for o: evicted per (head, q-tile) during the fused
            # loop (frees its SBUF for per-head q/k/v), streamed back in
            # phase 3
            o_dram = ccdram.tile([HPC, P, S], mdt)
            # persistent small tiles (cos on partitions 0:64, sin on 64:128)
            # -- loaded via gpsimd so the first xt/wt loads own the HWDGE
            # queues from t=0
            cs_sb = persist.tile([P, S], mdt)  # bf16 (gpsimd DMA casts)
            nc.gpsimd.dma_start(cs_sb[0:HH, :], cosT_d[:])
            nc.gpsimd.dma_start(cs_sb[HH:, :], sinT_d[:])
            tri_b = persist.tile([P, P], mdt)
            nc.gpsimd.dma_start(tri_b[:], tri_d[:])  # gpsimd DMA casts f32->bf16
            ident_b = persist.tile([P, P], mdt)
            make_identity(nc, ident_b)
            ones_f = persist.tile([P, 1], f32)
            nc.vector.memset(ones_f[:], 1.0)
            ones_c = persist.tile([P, 1], mdt)
            nc.vector.tensor_copy(ones_c[:], ones_f[:])
            ones_r1 = persist.tile([1, P], mdt)
            nc.vector.memset(ones_r1[:], 1.0)

            akT_all = persist.tile([P, HPC, AL], mdt)
            av_all = persist.tile([AL, HPC, P], mdt)
            nc.gpsimd.dma_start(akT_all[:], akT_d[:])
            nc.gpsimd.dma_start(av_all[:], av_d[:])

            # ============ Fused projections + attention, head-major ============
            # For each head: project q/k/v for all of S (x streamed per
            # 512-col tile, the 3 weight slices resident), rope in place,
            # then run the head's attention. The per-head ACT exp burst
            # overlaps the NEXT head's projection matmuls, so the PE never
            # waits on the softmax. o is staged straight to DRAM scratch
            # (no SBUF persistence); q/k/v only ever live per-head.
            #
            # scoresT layout [k, q]: p = exp(kT_blk.T @ qT_tile * inv_sqrt)
            # lands directly in the layout p@v needs -- no p transposes.
            # Scores are O(5) so exp needs no max subtraction; causal masking
            # multiplies the diagonal-band blocks by a 0/1 mask (on Pool).
            # Per-q sums first collapse 4 full blocks at a time into bf16
            # group tiles on the DVE (4x mode), so the PE ones-row matmul
            # streams 3.4x fewer rows. Normalization happens at eviction via
            # a K=1 broadcast matmul of 1/sums.
            HPC2 = HPC if 2 in phases else 0
            NXS = S // XS
            HC = DC // 2
            with ExitStack() as _stk:
                _pool = lambda **kw: _stk.enter_context(tc.tile_pool(**kw))
                hq = _pool(name="hq", bufs=2)
                hk = _pool(name="hk", bufs=2)
                hv = _pool(name="hv", bufs=2)
                p1x = _pool(name="p1x", bufs=2)
                p1w = _pool(name="p1w", bufs=4)
                p1t = _pool(name="p1t", bufs=1)
                p1v = _pool(name="p1v", bufs=2)
                p2pt = _pool(name="p2pt", bufs=cfg.pipe_depth + 1)
                p2pg = _pool(name="p2pg", bufs=cfg.pipe_depth + 1)
                p2sm = _pool(name="p2sm", bufs=3)
                p2rf = _pool(name="p2rf", bufs=1)
                p2ad = _pool(name="p2ad", bufs=2)
                p2o = _pool(name="p2o", bufs=2)
                p1ps = _pool(name="p1ps", bufs=2, space="PSUM")
                p2ps_s = _pool(name="p2ps_s", bufs=2, space="PSUM")
                p2ps_o = _pool(name="p2ps_o", bufs=1, space="PSUM")
                p2ps_t = _pool(name="p2ps_t", bufs=1, space="PSUM")
                p2ps_b = _pool(name="p2ps_b", bufs=2, space="PSUM")

                def issue_head_loads(h):
                    """Prefetch head h's weight slices + first x tile."""
                    wts = []
                    for pi, w_dram in enumerate((wq_d, wk_d, wv_d)):
                        wt = p1w.tile([P, DC, HD], mdt, tag="wt")
                        eng = nc.scalar if pi % 2 == 0 else nc.sync
                        eng.dma_start(wt[:], w_dram[:, h])
                        wts.append(wt)
                    xt0 = p1x.tile([P, DC, XS], mdt, tag="xt")
                    nc.sync.dma_start(xt0[:, 0:HC, :], xT_d[:, 0:HC, 0:XS])
                    nc.sync.dma_start(xt0[:, HC:, :], xT_d[:, HC:, 0:XS])
                    return wts, xt0

                def project_head(h, wts, xt0, q_cur, k_cur, v_nat, ps_a16):
                    akT = akT_all[:, h, :]
                    for st in range(NXS):
                        soff = st * XS
                        if st == 0:
                            xt = xt0
                        else:
                            xt = p1x.tile([P, DC, XS], mdt, tag="xt")
                            nc.sync.dma_start(
                                xt[:, 0:HC, :], xT_d[:, 0:HC, soff : soff + XS]
                            )
                            nc.sync.dma_start(
                                xt[:, HC:, :], xT_d[:, HC:, soff : soff + XS]
                            )
                        for pi, (proj, dst) in enumerate(
                            (("q", q_cur), ("k", k_cur), ("v", None))
                        ):
                            psum = p1ps.tile([P, XS], f32, tag="p1psum")
                            for c in range(DC):
                                nc.tensor.matmul(
                                    psum[:],
                                    wts[pi][:, c, :],
                                    xt[:, c, :],
                                    start=(c == 0),
                                    stop=(c == DC - 1),
                                )
                            if proj == "v":
                                # v to natural [s, d] layout via a DMA-XBAR
                                # transpose of each 512-col strip
                                v_stage = p1v.tile([P, XS], mdt, tag="v_stage")
                                nc.vector.tensor_copy(v_stage[:], psum[:])
                                nc.sync.dma_start(
                                    v_nat[:, st * 4 : (st + 1) * 4, :],
                                    v_stage[:],
                                    transpose=True,
                                )
                            else:
                                # rope: psum partitions 0:64 = even dims (x0),
                                # 64:128 = odd dims (x1). The four products go
                                # to base-0 tmp tiles (PSUM x SBUF inputs may
                                # differ in base partition; SBUF x SBUF may
                                # not), the two combines are base-aligned and
                                # write straight into the per-head q/k tile.
                                c_ap = cs_sb[0:HH, soff : soff + XS]
                                s_ap = cs_sb[HH:, soff : soff + XS]
                                x0 = psum[0:HH, :]
                                x1 = psum[HH : 2 * HH, :]
                                ta = p1t.tile([HH, XS], f32, tag="ta")
                                tb = p1t.tile([HH, XS], f32, tag="tb")
                                # second product pair reuses the first pair's
                                # buffers: DVE is in-order, the sub has read
                                # them by the time the tc2/td writes execute
                                tc2 = p1t.tile([HH, XS], f32, tag="ta")
                                td = p1t.tile([HH, XS], f32, tag="tb")
                                nc.vector.tensor_tensor(ta[:], x0, c_ap, op=Mul)
                                nc.vector.tensor_tensor(tb[:], x1, s_ap, op=Mul)
                                nc.vector.tensor_sub(
                                    dst[0:HH, soff : soff + XS], ta[:], tb[:]
                                )
                                nc.vector.tensor_tensor(tc2[:], x0, s_ap, op=Mul)
                                nc.vector.tensor_tensor(td[:], x1, c_ap, op=Mul)
                                nc.vector.tensor_add(
                                    dst[HH:, soff : soff + XS], tc2[:], td[:]
                                )
                                if proj == "q":
                                    # adapter scores for this strip's q blocks
                                    # (tiny matmuls; paying them here removes
                                    # the serial bubble at attention start)
                                    for qb in range(4):
                                        blk = st * 4 + qb
                                        nc.tensor.matmul(
                                            ps_a16[:, blk, :],
                                            q_cur[:, blk * P : (blk + 1) * P],
                                            akT[:],
                                            start=True,
                                            stop=True,
                                        )

                def emit_pv(ph, pQ, ptb, psg, apT, v_nat):
                    """sums + normalize-broadcast + p@v + adapter + evict.

                    The p@v matmuls are issued between the sums matmul and
                    the broadcast matmul so the DVE reciprocal overlaps PE
                    work instead of stalling it.
                    """
                    nkb = (pQ + 1) * 4
                    ps_su = p2ps_b.tile([1, 512], f32, tag="ps_b")
                    # sums: full-chunk group tiles + the 4 boundary blocks
                    # raw (their DVE pre-sum chain costs more than the PE
                    # rows here, and the PE has idle slack in phase 2)
                    nsu = pQ + 4
                    idx = 0
                    for g in range(pQ):
                        nc.tensor.matmul(
                            ps_su[:],
                            ones_c[:],
                            psg[:, g, :],
                            start=(idx == 0),
                            stop=False,
                        )
                        idx += 1
                    for j in range(4):
                        jb = 4 * pQ + j
                        off = j * P
                        nc.tensor.matmul(
                            ps_su[:, off:],
                            ones_c[:],
                            ptb[:, jb, off:],
                            start=(idx == 0),
                            stop=(idx == nsu - 1),
                        )
                        idx += 1
                    # fast approx reciprocal (~18 bits, 5x faster than the
                    # full-precision op whose ~3.3us latency stalled the
                    # broadcast matmul); result truncates to bf16 anyway
                    # bufs=1 is safe: the bf16 copy consumes rrow_f before
                    # the next emit's approx write (DVE is in-order)
                    rrow_f = p2rf.tile([1, 512], f32, tag="rrow_f")
                    nc.vector.reciprocal_approx_fast(rrow_f[:], ps_su[:])
                    rrow = p2sm.tile([1, 512], mdt, tag="rrow")
                    nc.vector.tensor_copy(rrow[:], rrow_f[:])
                    ps_o = p2ps_o.tile([P, 512], f32, tag="ps_o")
                    for jb in range(nkb):
                        off = 0 if jb < 4 * pQ else (jb - 4 * pQ) * P
                        nc.tensor.matmul(
                            ps_o[:, off:],
                            v_nat[:, jb, :],
                            ptb[:, jb, off:],
                            start=(jb == 0),
                            stop=(jb == nkb - 1),
                        )
                    ps_bc = p2ps_b.tile([P, 512], f32, tag="ps_b")
                    nc.tensor.matmul(
                        ps_bc[:], ones_r1[:], rrow[:], start=True, stop=True
                    )
                    ps_a2 = p2ps_b.tile([P, 512], f32, tag="ps_b")
                    nc.tensor.matmul(
                        ps_a2[:], av_all[:, ph, :], apT[:], start=True, stop=True
                    )
                    bc_sb = p2o.tile([P, 512], mdt, tag="bc_sb")
                    nc.any.tensor_copy(bc_sb[:], ps_bc[:])
                    o_ev = p2o.tile([P, 512], mdt, tag="o_ev")
                    nc.vector.scalar_tensor_tensor(
                        o_ev[:], ps_o[:], 1.0, bc_sb[:], op0=Mul, op1=Mul
                    )
                    nc.vector.tensor_add(o_ev[:], o_ev[:], ps_a2[:])
                    nc.gpsimd.dma_start(
                        o_dram[ph, :, pQ * 512 : (pQ + 1) * 512], o_ev[:]
                    )

                pending = []
                nxt = issue_head_loads(0) if HPC2 else None
                for h in range(HPC2):
                    # per-head q/k/v tiles; projections first, attention after
                    q_cur = hq.tile([P, S], mdt, tag="q_cur")
                    k_cur = hk.tile([P, S], mdt, tag="k_cur")
                    v_nat = hv.tile([P, NQ, P], mdt, tag="v_nat")
                    ps_a16 = p2ps_t.tile([P, NQ, AL], f32, tag="ps_t")
                    wts, xt0 = nxt
                    project_head(h, wts, xt0, q_cur, k_cur, v_nat, ps_a16)
                    if h + 1 < HPC2:
                        nxt = issue_head_loads(h + 1)

                    qT = q_cur
                    kT = k_cur

                    # adapter softmax chain (scores already accumulated
                    # during the projection pass)
                    asm16 = p2ad.tile([P, NQ, AL], f32, tag="asm")
                    nc.scalar.activation(
                        asm16[:], ps_a16[:], ExpF, bias=0.0, scale=inv_sqrt
                    )
                    asum16 = p2ad.tile([P, NQ], f32, tag="asum")
                    nc.vector.reduce_sum(out=asum16[:], in_=asm16[:], axis=AX)
                    arec16 = p2ad.tile([P, NQ], f32, tag="arec")
                    nc.vector.reciprocal_approx_fast(arec16[:], asum16[:])
                    asm16b = p2ad.tile([P, NQ, AL], mdt, tag="asmb")
                    nc.vector.tensor_tensor(
                        asm16b[:],
                        asm16[:],
                        arec16[:, :, None].to_broadcast([P, NQ, AL]),
                        op=Mul,
                    )

                    for Q in range(NT):
                        nkb = (Q + 1) * 4
                        nfull = 4 * Q
                        qtile = qT[:, Q * 512 : (Q + 1) * 512]
                        ptb = p2pt.tile([P, NQ, 512], mdt, tag="ptb")
                        psg = p2pg.tile([P, NT, 512], mdt, tag="psg")
                        apT = p2sm.tile([AL, 512], mdt, tag="apT")
                        # full k-blocks: one score matmul + exp per block
                        # (single-bank psum tiles keep the fused-phase PSUM
                        # budget inside 8 banks)
                        for jb in range(nfull):
                            ps_s = p2ps_s.tile([P, 512], f32, tag="ps_s")
                            nc.tensor.matmul(
                                ps_s[:],
                                kT[:, jb * P : (jb + 1) * P],
                                qtile[:],
                                start=True,
                                stop=True,
                            )
                            nc.scalar.activation(
                                ptb[:, jb, :],
                                ps_s[:],
                                ExpF,
                                bias=0.0,
                                scale=inv_sqrt,
                            )
                        # boundary band: 4 blocks, each exp'd from its own
                        # diagonal offset; triangle mask on Pool
                        for bj in range(4):
                            jb = nfull + bj
                            off = bj * P
                            ps_s = p2ps_s.tile([P, 512], f32, tag="ps_s")
                            nc.tensor.matmul(
                                ps_s[:, off:],
                                kT[:, jb * P : (jb + 1) * P],
                                qtile[:, off:],
                                start=True,
                                stop=True,
                            )
                            nc.scalar.activation(
                                ptb[:, jb, off:],
                                ps_s[:, off:],
                                ExpF,
                                bias=0.0,
                                scale=inv_sqrt,
                            )
                            nc.gpsimd.tensor_mul(
                                ptb[:, jb, off : off + P],
                                ptb[:, jb, off : off + P],
                                tri_b[:],
                            )
                        # grouped block sums: 4 full blocks per group, on the
                        # lightly-loaded Pool engine (SBUF-only operands) so
                        # the DVE queue drains before the next projection
                        # block's rope ops need it; boundary blocks are
                        # summed raw by the PE ones-matmul in emit_pv
                        for g in range(Q):
                            b = 4 * g
                            nc.gpsimd.tensor_add(
                                psg[:, g, :], ptb[:, b, :], ptb[:, b + 1, :]
                            )
                            nc.gpsimd.tensor_add(
                                psg[:, g, :], psg[:, g, :], ptb[:, b + 2, :]
                            )
                            nc.gpsimd.tensor_add(
                                psg[:, g, :], psg[:, g, :], ptb[:, b + 3, :]
                            )
                        # pipeline: heavy tail of an OLDER q-tile before the
                        # adapter chain, so PE stays fed while the newer
                        # tile's exps run on ACT
                        if len(pending) >= cfg.pipe_depth:
                            emit_pv(*pending.pop(0))

                        # adapter probs for this q tile: transpose the
                        # head-level normalized probs into [AL, q]
                        for qb in range(4):
                            ps_apt = p2ps_t.tile([P, P], mdt, tag="ps_t")
                            nc.tensor.transpose(
                                ps_apt[:AL, :],
                                asm16b[:, Q * 4 + qb, :],
                                ident_b[:],
                            )
                            nc.any.tensor_copy(
                                apT[:, qb * P : (qb + 1) * P], ps_apt[:AL, :]
                            )
                        pending.append((h, Q, ptb, psg, apT, v_nat))
                for entry in pending:
                    emit_pv(*entry)
                pending = []

            # ================= Phase 3: out @ wo =================
            # Full-width [S, D] partial over this core's 8 heads; the host
            # sums the 4 TP partials per batch (the reduction is free there).
            # o streams back from DRAM scratch in 512-row s-groups on the
            # idle Pool queue; all 8 wo tiles stay resident across the s loop.
            with (
                tc.tile_pool(name="p3w", bufs=1) as p3w,
                tc.tile_pool(name="p3o", bufs=2) as p3o,
                tc.tile_pool(name="p3y", bufs=3) as p3y,
                tc.tile_pool(name="p3ps", bufs=4, space="PSUM") as p3ps,
            ):
                wo_ts = []
                for et in range(ET3 if 3 in phases else 0):
                    wo_t = p3w.tile(
                        [P, WC3, 512], mdt, name=f"wo_t{et}", tag=f"wo_t{et}"
                    )
                    eng = nc.sync if et % 2 == 0 else nc.scalar
                    eng.dma_start(wo_t[:], wo_d[:, et])
                    wo_ts.append(wo_t)
                for sg in range(NQ // 4 if 3 in phases else 0):
                    o_sg = p3o.tile([P, HPC, 512], mdt, tag="o_sg")
                    for h in range(HPC):
                        nc.gpsimd.dma_start(
                            o_sg[:, h, :],
                            o_dram[h, :, sg * 512 : (sg + 1) * 512],
                        )
                    for st4 in range(4):
                        for et in range(ET3):
                            ps_y = p3ps.tile([P, 512], f32, tag="ps_y")
                            for w in range(WC3):
                                nc.tensor.matmul(
                                    ps_y[:],
                                    o_sg[:, w, st4 * P : (st4 + 1) * P],
                                    wo_ts[et][:, w, :],
                                    start=(w == 0),
                                    stop=(w == WC3 - 1),
                                )
                            y_sb = p3y.tile([P, 512], f32, tag="y_sb")
                            nc.scalar.copy(y_sb[:], ps_y[:])
                            st = sg * 4 + st4
                            nc.sync.dma_start(
                                y_d[st * P : (st + 1) * P,
                                    et * 512 : (et + 1) * 512],
                                y_sb[:],
                            )

    nc.compile()
    return nc


# ====================== host side: sharding + runner ======================

B, S, D, H = 2, 2048, 4096, 32
HD = D // H
AL = 10
N_CORES = 8
TP = 4  # head groups
HPC = H // TP  # 8 heads per core

_RUNNER = None


def _make_runner(nc, n_cores=N_CORES):
    import jax
    from jax.sharding import Mesh, PartitionSpec
    from jax.experimental.shard_map import shard_map

    from concourse import bass2jax
    from concourse.bass2jax import _bass_exec_p, install_neuronx_cc_hook

    install_neuronx_cc_hook()
    partition_name = nc.partition_id_tensor.name if nc.partition_id_tensor else None

    in_names, out_names, out_avals = [], [], []
    for alloc in nc.m.functions[0].allocations:
        if not isinstance(alloc, mybir.MemoryLocationSet):
            continue
        name = alloc.memorylocations[0].name
        if alloc.kind == "ExternalInput":
            if name != partition_name:
                in_names.append(name)
        elif alloc.kind == "ExternalOutput":
            out_names.append(name)
            out_avals.append(
                jax.core.ShapedArray(
                    tuple(alloc.tensor_shape), mybir.dt.np(alloc.dtype)
                )
            )
    n_params = len(in_names)
    n_outs = len(out_avals)
    all_in_names = list(in_names) + list(out_names)
    if partition_name is not None:
        all_in_names.append(partition_name)

    def _body(*args):
        operands = list(args)
        if partition_name is not None:
            operands.append(bass2jax.partition_id_tensor())
        outs = _bass_exec_p.bind(
            *operands,
            out_avals=tuple(out_avals),
            in_names=tuple(all_in_names),
            out_names=tuple(out_names),
            lowering_input_output_aliases=(),
            sim_require_finite=True,
            sim_require_nnan=True,
            nc=nc,
        )
        return tuple(outs)

    devices = jax.devices()[:n_cores]
    mesh = Mesh(np.asarray(devices), ("core",))
    fn = jax.jit(
        shard_map(
            _body,
            mesh=mesh,
            in_specs=(PartitionSpec("core"),) * (n_params + n_outs),
            out_specs=(PartitionSpec("core"),) * n_outs,
            check_rep=False,
        ),
        keep_unused=True,
    )

    class Runner:
        in_names_ = in_names
        out_names_ = out_names

        def prep(self, in_maps):
            import jax as _jax

            concat_in = [
                np.concatenate(
                    [np.ascontiguousarray(in_maps[c][n]) for c in range(n_cores)],
                    axis=0,
                )
                for n in in_names
            ]
            concat_zero = [
                np.zeros((n_cores * a.shape[0], *a.shape[1:]), a.dtype)
                for a in out_avals
            ]
            shardings = [
                _jax.sharding.NamedSharding(mesh, PartitionSpec("core"))
            ] * (n_params + n_outs)
            return _jax.device_put(concat_in + concat_zero, shardings)

        def run(self, args):
            import jax as _jax

            outs = fn(*args)
            _jax.block_until_ready(outs)
            return [
                {
                    n: np.asarray(outs[i]).reshape(n_cores, *out_avals[i].shape)[c]
                    for i, n in enumerate(out_names)
                }
                for c in range(n_cores)
            ]

        def time_pipelined(self, args, reps=10, warmup=1):
            import time as _time

            import jax as _jax

            for _ in range(warmup):
                _jax.block_until_ready(fn(*args))
            t0 = _time.perf_counter()
            outs = None
            for _ in range(reps):
                outs = fn(*args)
            _jax.block_until_ready(outs)
            return (_time.perf_counter() - t0) / reps

    return Runner()


def _shard_inputs(x, cos, sin, mask, wq, wk, wv, wo, gate, adapter):
    """Build the 8 per-core input maps."""
    import ml_dtypes

    bf = ml_dtypes.bfloat16
    # rope permutation of head-dim columns: even dims first, odd second
    perm = np.concatenate(
        [np.arange(0, HD, 2), np.arange(1, HD, 2)]
    )  # within one head
    col_perm = np.concatenate(
        [h * HD + perm for h in range(H)]
    )  # all heads, head-major
    wq_f = np.asarray(wq, dtype=np.float32)[:, col_perm]
    wk_f = np.asarray(wk, dtype=np.float32)[:, col_perm]
    wv_f = np.asarray(wv, dtype=np.float32)
    wq_p = wq_f.astype(bf)
    wk_p = wk_f.astype(bf)
    wv_b = wv_f.astype(bf)
    wo_b = np.asarray(wo, dtype=np.float32).astype(bf)

    DC = D // P
    HPC_ = HPC
    WC3 = HPC_  # phase-3 contraction chunks (own heads)
    ET3 = D // 512  # full-width out tiles

    def _prearrange_w(w_slice):
        # [D, DH] -> [P, HPC, DC, HD]: contiguous per-partition head tiles
        return np.ascontiguousarray(
            w_slice.reshape(DC, P, HPC_, HD).transpose(1, 2, 0, 3)
        )

    def _prearrange_wo(wo_slice):
        # [DH, D] -> [P, ET3, WC3, 512]
        return np.ascontiguousarray(
            wo_slice.reshape(WC3, P, ET3, 512).transpose(1, 2, 0, 3)
        )

    cosT = np.ascontiguousarray(cos.T, dtype=np.float32)  # [64, S]
    sinT = np.ascontiguousarray(sin.T, dtype=np.float32)

    # 0/1 [k, q] allowed-mask of an aligned 128x128 diagonal block,
    # derived from the mask input (k <= q allowed)
    m = np.asarray(mask, dtype=np.float32)[0, 0]  # [S, S]
    tri = np.ascontiguousarray((m[:P, :P].T == 0)).astype(np.float32)

    gate_v = np.asarray(gate, dtype=np.float32).reshape(H)  # per head
    ad_f = np.asarray(adapter, dtype=np.float32)[0]  # [AL, D]

    xT = [
        np.ascontiguousarray(
            np.asarray(x[b], dtype=np.float32)
            .T.astype(bf)
            .reshape(D // P, P, S)
            .transpose(1, 0, 2)
        )
        for b in range(B)
    ]  # [P, DC, S]

    in_maps = []
    for c in range(N_CORES):
        b = c // TP
        g = c % TP
        hs = g * HPC * HD  # column slice start

        # host-computed adapter projections for this core's heads
        ak = ad_f @ wk_f[:, hs : hs + HPC * HD]  # [AL, HPC*HD], rope basis
        akT = np.ascontiguousarray(
            ak.reshape(AL, HPC_, HD).transpose(2, 1, 0)
        ).astype(bf)  # [P(hd), HPC, AL]
        av = ad_f @ wv_f[:, hs : hs + HPC * HD]  # [AL, HPC*HD]
        av = av.reshape(AL, HPC_, HD) * gate_v[g * HPC : (g + 1) * HPC][None, :, None]
        av = np.ascontiguousarray(av).astype(bf)  # [AL, HPC, P]

        in_maps.append(
            {
                "xT": xT[b],
                "wq": _prearrange_w(wq_p[:, hs : hs + HPC * HD]),
                "wk": _prearrange_w(wk_p[:, hs : hs + HPC * HD]),
                "wv": _prearrange_w(wv_b[:, hs : hs + HPC * HD]),
                "wo": _prearrange_wo(wo_b[hs : hs + HPC * HD, :]),
                "akT": akT,
                "av": av,
                "cosT": cosT,
                "sinT": sinT,
                "tri": tri,
            }
        )
    return in_maps


def get_runner():
    global _RUNNER
    if _RUNNER is None:
        nc = build_nc(Cfg())
        _RUNNER = _make_runner(nc)
    return _RUNNER


def kernel(**inputs) -> np.ndarray:
    x = np.asarray(inputs["x"])
    in_maps = _shard_inputs(
        x,
        inputs["cos"],
        inputs["sin"],
        inputs["mask"],
        inputs["wq"],
        inputs["wk"],
        inputs["wv"],
        inputs["wo"],
        inputs["gate"],
        inputs["adapter"],
    )
    runner = get_runner()
    args = runner.prep(in_maps)
    outs = runner.run(args)
    y = np.zeros((B, S, D), dtype=np.float32)
    for c in range(N_CORES):
        y[c // TP] += outs[c]["y"]
    return y



# revision 50
# speedup vs baseline: 1.7474x; 1.7474x over previous
"""Trainium2 Bass kernel for nn_Attention_66563403153646.

Dense transformer attention block with rotary embeddings + gated adapter
(prefix) attention, fp32 reference:

    y = softmax(rope(x@wq) @ rope(x@wk).T * k + mask) @ (x@wv)
      + gate * softmax(rope(x@wq) @ (adapter@wk).T * k) @ (adapter@wv)
    out = y @ wo

Sharding: 4-way tensor-parallel over heads x 2-way data-parallel over batch
(8 NeuronCores). Each core computes a [S, D] partial of its batch's output
(its 8 heads' contribution through wo); the host sums the 4 TP partials
(that reduction is free host-side, and the o@wo work per core is invariant
to any other split of the contraction across the TP group).

All matmul operands are bf16 (PE 1 cycle/row); accumulation is fp32 in
PSUM. The kernel is PE-cycle-bound, so the schedule is a fused per-HEAD
pipeline: project q/k/v for head h (x streamed in 512-col tiles, the three
weight slices resident), rope in place, then run head h's attention while
head h+1's inputs prefetch -- the per-head softmax exp burst on the ACT
engine hides under the next head's projection matmuls, and q/k/v only ever
need per-head SBUF tiles. Layouts:
  - x is fed transposed ([D, S]) so projections contract D on partitions.
  - q/k live per-head in [HD, S] layout with rope-pair-permuted head dims
    (host permutes wq/wk columns: even rope dims first, odd second) so
    RoPE is two partition-halves of elementwise ops.
  - v is transposed to natural [s, d] per 512-col strip via DMA-XBAR.
  - scores are computed per 512-col q tile over 128-row k blocks
    (causal: only blocks <= diagonal; the diagonal band gets a 0/1
    triangle mask on the Pool engine).
  - p blocks land in [k, q] layout, which is exactly what p@v needs; the
    normalized o evicts to DRAM scratch per (head, q-tile) and streams
    back for the wo pass in 512-row groups.
  - adapter projections (ak/av, gate folded into av) are precomputed on
    the host (tiny: 10 x D x 2DH); the per-q-block adapter score matmuls
    ride inside the projection pass.
"""

import sys

sys.path.insert(0, "/opt/trn_rl_repo")

import math
from contextlib import ExitStack
from dataclasses import dataclass

import numpy as np

import concourse.bass as bass
import concourse.mybir as mybir
import concourse.tile as tile
from concourse import bacc
from concourse.masks import make_identity

f32 = mybir.dt.float32
f32r = mybir.dt.float32r
bf16 = mybir.dt.bfloat16

P = 128


@dataclass(frozen=True)
class Cfg:
    S: int = 2048  # sequence length
    D: int = 4096  # model dim
    HPC: int = 8  # heads per core
    HD: int = 128  # head dim
    AL: int = 10  # adapter len
    mm_dt: object = bf16  # matmul operand dtype
    pipe_depth: int = 2  # phase-2 software pipeline depth

    @property
    def DC(self):  # D chunks of 128 (contraction)
        return self.D // P

    @property
    def DH(self):  # head-slice width
        return self.HPC * self.HD

    @property
    def NQ(self):  # 128-row q blocks
        return self.S // P

    @property
    def NT(self):  # 512-col tiles
        return self.S // 512

    @property
    def XS(self):  # phase-1 x s-tile width
        return 512


def build_nc(cfg: Cfg, phases=(1, 2, 3)):
    nc = bacc.Bacc(None, target_bir_lowering=False, debug=False, num_devices=8)
    S, D, HPC, HD, AL = cfg.S, cfg.D, cfg.HPC, cfg.HD, cfg.AL
    DC, DH, NQ, NT, XS = cfg.DC, cfg.DH, cfg.NQ, cfg.NT, cfg.XS
    mdt = cfg.mm_dt
    inv_sqrt = 1.0 / math.sqrt(HD)
    HH = HD // 2
    WC3 = HPC  # phase-3 contraction chunks (own heads)
    ET3 = D // 512  # full-width output tiles; host sums the 4 TP partials

    # ---- I/O ----
    # All big operands arrive host-pre-arranged in partition-major layouts
    # so every DMA is one large contiguous line per partition.
    xT_d = nc.dram_tensor("xT", [P, DC, S], mdt, kind="ExternalInput")
    wq_d = nc.dram_tensor("wq", [P, HPC, DC, HD], mdt, kind="ExternalInput")
    wk_d = nc.dram_tensor("wk", [P, HPC, DC, HD], mdt, kind="ExternalInput")
    wv_d = nc.dram_tensor("wv", [P, HPC, DC, HD], mdt, kind="ExternalInput")
    wo_d = nc.dram_tensor("wo", [P, ET3, WC3, 512], mdt, kind="ExternalInput")
    # host-computed adapter projections (tiny): akT in rope-permuted basis,
    # av with the per-head gate folded in
    akT_d = nc.dram_tensor("akT", [P, HPC, AL], mdt, kind="ExternalInput")
    av_d = nc.dram_tensor("av", [AL, HPC, P], mdt, kind="ExternalInput")
    cosT_d = nc.dram_tensor("cosT", [HH, S], f32, kind="ExternalInput")
    sinT_d = nc.dram_tensor("sinT", [HH, S], f32, kind="ExternalInput")
    # 0/1 upper-triangular (incl diag) [k,q] mask for the boundary block
    tri_d = nc.dram_tensor("tri", [P, P], f32, kind="ExternalInput")
    y_d = nc.dram_tensor("y", [S, D], f32, kind="ExternalOutput")

    ExpF = mybir.ActivationFunctionType.Exp
    AX = mybir.AxisListType.X
    Mul = mybir.AluOpType.mult

    with tile.TileContext(nc) as tc:
        with (
            tc.tile_pool(name="persist", bufs=1) as persist,
            tc.tile_pool(name="ccdram", bufs=1, space="DRAM") as ccdram,
        ):
            # DRAM scratch for o: evicted per (head, q-tile) during the fused
            # loop (frees its SBUF for per-head q/k/v), streamed back in
            # phase 3
            o_dram = ccdram.tile([HPC, P, S], mdt)
            # persistent small tiles (cos on partitions 0:64, sin on 64:128)
            # -- loaded via gpsimd so the first xt/wt loads own the HWDGE
            # queues from t=0
            cs_sb = persist.tile([P, S], mdt)  # bf16 (gpsimd DMA casts)
            nc.gpsimd.dma_start(cs_sb[0:HH, :], cosT_d[:])
            nc.gpsimd.dma_start(cs_sb[HH:, :], sinT_d[:])
            tri_b = persist.tile([P, P], mdt)
            nc.gpsimd.dma_start(tri_b[:], tri_d[:])  # gpsimd DMA casts f32->bf16
            ident_b = persist.tile([P, P], mdt)
            make_identity(nc, ident_b)
            ones_f = persist.tile([P, 1], f32)
            nc.vector.memset(ones_f[:], 1.0)
            ones_c = persist.tile([P, 1], mdt)
            nc.vector.tensor_copy(ones_c[:], ones_f[:])
            ones_r1 = persist.tile([1, P], mdt)
            nc.vector.memset(ones_r1[:], 1.0)

            akT_all = persist.tile([P, HPC, AL], mdt)
            av_all = persist.tile([AL, HPC, P], mdt)
            nc.gpsimd.dma_start(akT_all[:], akT_d[:])
            nc.gpsimd.dma_start(av_all[:], av_d[:])

            # ============ Fused projections + attention, head-major ============
            # For each head: project q/k/v for all of S (x streamed per
            # 512-col tile, the 3 weight slices resident), rope in place,
            # then run the head's attention. The per-head ACT exp burst
            # overlaps the NEXT head's projection matmuls, so the PE never
            # waits on the softmax. o is staged straight to DRAM scratch
            # (no SBUF persistence); q/k/v only ever live per-head.
            #
            # scoresT layout [k, q]: p = exp(kT_blk.T @ qT_tile * inv_sqrt)
            # lands directly in the layout p@v needs -- no p transposes.
            # Scores are O(5) so exp needs no max subtraction; causal masking
            # multiplies the diagonal-band blocks by a 0/1 mask (on Pool).
            # Per-q sums first collapse 4 full blocks at a time into bf16
            # group tiles on the DVE (4x mode), so the PE ones-row matmul
            # streams 3.4x fewer rows. Normalization happens at eviction via
            # a K=1 broadcast matmul of 1/sums.
            HPC2 = HPC if 2 in phases else 0
            NXS = S // XS
            HC = DC // 2
            with ExitStack() as _stk:
                _pool = lambda **kw: _stk.enter_context(tc.tile_pool(**kw))
                hq = _pool(name="hq", bufs=2)
                hk = _pool(name="hk", bufs=2)
                hv = _pool(name="hv", bufs=2)
                p1x = _pool(name="p1x", bufs=2)
                p1w = _pool(name="p1w", bufs=4)
                p1t = _pool(name="p1t", bufs=1)
                p1v = _pool(name="p1v", bufs=2)
                p2pt = _pool(name="p2pt", bufs=cfg.pipe_depth + 1)
                p2pg = _pool(name="p2pg", bufs=cfg.pipe_depth + 1)
                p2sm = _pool(name="p2sm", bufs=3)
                p2rf = _pool(name="p2rf", bufs=1)
                p2ad = _pool(name="p2ad", bufs=2)
                p2o = _pool(name="p2o", bufs=2)
                p1ps = _pool(name="p1ps", bufs=2, space="PSUM")
                p2ps_s = _pool(name="p2ps_s", bufs=2, space="PSUM")
                p2ps_o = _pool(name="p2ps_o", bufs=1, space="PSUM")
                p2ps_t = _pool(name="p2ps_t", bufs=1, space="PSUM")
                p2ps_b = _pool(name="p2ps_b", bufs=2, space="PSUM")

                def issue_head_loads(h):
                    """Prefetch head h's weight slices + first x tile."""
                    wts = []
                    for pi, w_dram in enumerate((wq_d, wk_d, wv_d)):
                        wt = p1w.tile([P, DC, HD], mdt, tag="wt")
                        eng = nc.scalar if pi % 2 == 0 else nc.sync
                        eng.dma_start(wt[:], w_dram[:, h])
                        wts.append(wt)
                    xt0 = p1x.tile([P, DC, XS], mdt, tag="xt")
                    nc.sync.dma_start(xt0[:, 0:HC, :], xT_d[:, 0:HC, 0:XS])
                    nc.sync.dma_start(xt0[:, HC:, :], xT_d[:, HC:, 0:XS])
                    return wts, xt0

                def project_head(h, wts, xt0, q_cur, k_cur, v_nat, ps_a16):
                    akT = akT_all[:, h, :]
                    for st in range(NXS):
                        soff = st * XS
                        if st == 0:
                            xt = xt0
                        else:
                            xt = p1x.tile([P, DC, XS], mdt, tag="xt")
                            nc.sync.dma_start(
                                xt[:, 0:HC, :], xT_d[:, 0:HC, soff : soff + XS]
                            )
                            nc.sync.dma_start(
                                xt[:, HC:, :], xT_d[:, HC:, soff : soff + XS]
                            )
                        for pi, (proj, dst) in enumerate(
                            (("q", q_cur), ("k", k_cur), ("v", None))
                        ):
                            psum = p1ps.tile([P, XS], f32, tag="p1psum")
                            for c in range(DC):
                                nc.tensor.matmul(
                                    psum[:],
                                    wts[pi][:, c, :],
                                    xt[:, c, :],
                                    start=(c == 0),
                                    stop=(c == DC - 1),
                                )
                            if proj == "v":
                                # v to natural [s, d] layout via a DMA-XBAR
                                # transpose of each 512-col strip
                                v_stage = p1v.tile([P, XS], mdt, tag="v_stage")
                                nc.vector.tensor_copy(v_stage[:], psum[:])
                                nc.sync.dma_start(
                                    v_nat[:, st * 4 : (st + 1) * 4, :],
                                    v_stage[:],
                                    transpose=True,
                                )
                            else:
                                # rope: psum partitions 0:64 = even dims (x0),
                                # 64:128 = odd dims (x1). The four products go
                                # to base-0 tmp tiles (PSUM x SBUF inputs may
                                # differ in base partition; SBUF x SBUF may
                                # not), the two combines are base-aligned and
                                # write straight into the per-head q/k tile.
                                c_ap = cs_sb[0:HH, soff : soff + XS]
                                s_ap = cs_sb[HH:, soff : soff + XS]
                                x0 = psum[0:HH, :]
                                x1 = psum[HH : 2 * HH, :]
                                ta = p1t.tile([HH, XS], f32, tag="ta")
                                tb = p1t.tile([HH, XS], f32, tag="tb")
                                # second product pair reuses the first pair's
                                # buffers: DVE is in-order, the sub has read
                                # them by the time the tc2/td writes execute
                                tc2 = p1t.tile([HH, XS], f32, tag="ta")
                                td = p1t.tile([HH, XS], f32, tag="tb")
                                nc.vector.tensor_tensor(ta[:], x0, c_ap, op=Mul)
                                nc.vector.tensor_tensor(tb[:], x1, s_ap, op=Mul)
                                nc.vector.tensor_sub(
                                    dst[0:HH, soff : soff + XS], ta[:], tb[:]
                                )
                                nc.vector.tensor_tensor(tc2[:], x0, s_ap, op=Mul)
                                nc.vector.tensor_tensor(td[:], x1, c_ap, op=Mul)
                                nc.vector.tensor_add(
                                    dst[HH:, soff : soff + XS], tc2[:], td[:]
                                )
                                if proj == "q":
                                    # adapter scores for this strip's q blocks
                                    # (tiny matmuls; paying them here removes
                                    # the serial bubble at attention start)
                                    for qb in range(4):
                                        blk = st * 4 + qb
                                        nc.tensor.matmul(
                                            ps_a16[:, blk, :],
                                            q_cur[:, blk * P : (blk + 1) * P],
                                            akT[:],
                                            start=True,
                                            stop=True,
                                        )

                def emit_pv(ph, pQ, ptb, psg, apT, v_nat):
                    """sums + normalize-broadcast + p@v + adapter + evict.

                    The p@v matmuls are issued between the sums matmul and
                    the broadcast matmul so the DVE reciprocal overlaps PE
                    work instead of stalling it.
                    """
                    nkb = (pQ + 1) * 4
                    ps_su = p2ps_b.tile([1, 512], f32, tag="ps_b")
                    # sums: full-chunk group tiles + the 4 boundary blocks
                    # raw (their DVE pre-sum chain costs more than the PE
                    # rows here, and the PE has idle slack in phase 2)
                    nsu = pQ + 4
                    idx = 0
                    for g in range(pQ):
                        nc.tensor.matmul(
                            ps_su[:],
                            ones_c[:],
                            psg[:, g, :],
                            start=(idx == 0),
                            stop=False,
                        )
                        idx += 1
                    for j in range(4):
                        jb = 4 * pQ + j
                        off = j * P
                        nc.tensor.matmul(
                            ps_su[:, off:],
                            ones_c[:],
                            ptb[:, jb, off:],
                            start=(idx == 0),
                            stop=(idx == nsu - 1),
                        )
                        idx += 1
                    # fast approx reciprocal (~18 bits, 5x faster than the
                    # full-precision op whose ~3.3us latency stalled the
                    # broadcast matmul); result truncates to bf16 anyway
                    # bufs=1 is safe: the bf16 copy consumes rrow_f before
                    # the next emit's approx write (DVE is in-order)
                    rrow_f = p2rf.tile([1, 512], f32, tag="rrow_f")
                    nc.vector.reciprocal_approx_fast(rrow_f[:], ps_su[:])
                    rrow = p2sm.tile([1, 512], mdt, tag="rrow")
                    nc.vector.tensor_copy(rrow[:], rrow_f[:])
                    ps_o = p2ps_o.tile([P, 512], f32, tag="ps_o")
                    for jb in range(nkb):
                        off = 0 if jb < 4 * pQ else (jb - 4 * pQ) * P
                        nc.tensor.matmul(
                            ps_o[:, off:],
                            v_nat[:, jb, :],
                            ptb[:, jb, off:],
                            start=(jb == 0),
                            stop=(jb == nkb - 1),
                        )
                    ps_bc = p2ps_b.tile([P, 512], f32, tag="ps_b")
                    nc.tensor.matmul(
                        ps_bc[:], ones_r1[:], rrow[:], start=True, stop=True
                    )
                    ps_a2 = p2ps_b.tile([P, 512], f32, tag="ps_b")
                    nc.tensor.matmul(
                        ps_a2[:], av_all[:, ph, :], apT[:], start=True, stop=True
                    )
                    bc_sb = p2o.tile([P, 512], mdt, tag="bc_sb")
                    nc.any.tensor_copy(bc_sb[:], ps_bc[:])
                    o_ev = p2o.tile([P, 512], mdt, tag="o_ev")
                    nc.vector.scalar_tensor_tensor(
                        o_ev[:], ps_o[:], 1.0, bc_sb[:], op0=Mul, op1=Mul
                    )
                    nc.vector.tensor_add(o_ev[:], o_ev[:], ps_a2[:])
                    nc.gpsimd.dma_start(
                        o_dram[ph, :, pQ * 512 : (pQ + 1) * 512], o_ev[:]
                    )

                pending = []
                nxt = issue_head_loads(0) if HPC2 else None
                for h in range(HPC2):
                    # per-head q/k/v tiles; projections first, attention after
                    q_cur = hq.tile([P, S], mdt, tag="q_cur")
                    k_cur = hk.tile([P, S], mdt, tag="k_cur")
                    v_nat = hv.tile([P, NQ, P], mdt, tag="v_nat")
                    ps_a16 = p2ps_t.tile([P, NQ, AL], f32, tag="ps_t")
                    wts, xt0 = nxt
                    project_head(h, wts, xt0, q_cur, k_cur, v_nat, ps_a16)
                    if h + 1 < HPC2:
                        nxt = issue_head_loads(h + 1)

                    qT = q_cur
                    kT = k_cur

                    # adapter softmax chain (scores already accumulated
                    # during the projection pass)
                    asm16 = p2ad.tile([P, NQ, AL], f32, tag="asm")
                    nc.scalar.activation(
                        asm16[:], ps_a16[:], ExpF, bias=0.0, scale=inv_sqrt
                    )
                    asum16 = p2ad.tile([P, NQ], f32, tag="asum")
                    nc.vector.reduce_sum(out=asum16[:], in_=asm16[:], axis=AX)
                    arec16 = p2ad.tile([P, NQ], f32, tag="arec")
                    nc.vector.reciprocal_approx_fast(arec16[:], asum16[:])
                    asm16b = p2ad.tile([P, NQ, AL], mdt, tag="asmb")
                    nc.vector.tensor_tensor(
                        asm16b[:],
                        asm16[:],
                        arec16[:, :, None].to_broadcast([P, NQ, AL]),
                        op=Mul,
                    )

                    for Q in range(NT):
                        nkb = (Q + 1) * 4
                        nfull = 4 * Q
                        qtile = qT[:, Q * 512 : (Q + 1) * 512]
                        ptb = p2pt.tile([P, NQ, 512], mdt, tag="ptb")
                        psg = p2pg.tile([P, NT, 512], mdt, tag="psg")
                        apT = p2sm.tile([AL, 512], mdt, tag="apT")
                        # full k-blocks: one score matmul + exp per block
                        # (single-bank psum tiles keep the fused-phase PSUM
                        # budget inside 8 banks)
                        for jb in range(nfull):
                            ps_s = p2ps_s.tile([P, 512], f32, tag="ps_s")
                            nc.tensor.matmul(
                                ps_s[:],
                                kT[:, jb * P : (jb + 1) * P],
                                qtile[:],
                                start=True,
                                stop=True,
                            )
                            nc.scalar.activation(
                                ptb[:, jb, :],
                                ps_s[:],
                                ExpF,
                                bias=0.0,
                                scale=inv_sqrt,
                            )
                        # boundary band: 4 blocks, each exp'd from its own
                        # diagonal offset; triangle mask on Pool
                        for bj in range(4):
                            jb = nfull + bj
                            off = bj * P
                            ps_s = p2ps_s.tile([P, 512], f32, tag="ps_s")
                            nc.tensor.matmul(
                                ps_s[:, off:],
                                kT[:, jb * P : (jb + 1) * P],
                                qtile[:, off:],
                                start=True,
                                stop=True,
                            )
                            nc.scalar.activation(
                                ptb[:, jb, off:],
                                ps_s[:, off:],
                                ExpF,
                                bias=0.0,
                                scale=inv_sqrt,
                            )
                            nc.gpsimd.tensor_mul(
                                ptb[:, jb, off : off + P],
                                ptb[:, jb, off : off + P],
                                tri_b[:],
                            )
                        # grouped block sums (bf16, DVE 4x): 4 full blocks
                        # per group; boundary blocks are summed raw by the
                        # PE ones-matmul in emit_pv
                        for g in range(Q):
                            b = 4 * g
                            nc.vector.tensor_add(
                                psg[:, g, :], ptb[:, b, :], ptb[:, b + 1, :]
                            )
                            nc.vector.tensor_add(
                                psg[:, g, :], psg[:, g, :], ptb[:, b + 2, :]
                            )
                            nc.vector.tensor_add(
                                psg[:, g, :], psg[:, g, :], ptb[:, b + 3, :]
                            )
                        # pipeline: heavy tail of an OLDER q-tile before the
                        # adapter chain, so PE stays fed while the newer
                        # tile's exps run on ACT
                        if len(pending) >= cfg.pipe_depth:
                            emit_pv(*pending.pop(0))

                        # adapter probs for this q tile: transpose the
                        # head-level normalized probs into [AL, q]
                        for qb in range(4):
                            ps_apt = p2ps_t.tile([P, P], mdt, tag="ps_t")
                            nc.tensor.transpose(
                                ps_apt[:AL, :],
                                asm16b[:, Q * 4 + qb, :],
                                ident_b[:],
                            )
                            nc.any.tensor_copy(
                                apT[:, qb * P : (qb + 1) * P], ps_apt[:AL, :]
                            )
                        pending.append((h, Q, ptb, psg, apT, v_nat))
                for entry in pending:
                    emit_pv(*entry)
                pending = []

            # ================= Phase 3: out @ wo =================
            # Full-width [S, D] partial over this core's 8 heads; the host
            # sums the 4 TP partials per batch (the reduction is free there).
            # o streams back from DRAM scratch in 512-row s-groups on the
            # idle Pool queue; all 8 wo tiles stay resident across the s loop.
            with (
                tc.tile_pool(name="p3w", bufs=1) as p3w,
                tc.tile_pool(name="p3o", bufs=2) as p3o,
                tc.tile_pool(name="p3y", bufs=3) as p3y,
                tc.tile_pool(name="p3ps", bufs=4, space="PSUM") as p3ps,
            ):
                wo_ts = []
                for et in range(ET3 if 3 in phases else 0):
                    wo_t = p3w.tile(
                        [P, WC3, 512], mdt, name=f"wo_t{et}", tag=f"wo_t{et}"
                    )
                    eng = nc.sync if et % 2 == 0 else nc.scalar
                    eng.dma_start(wo_t[:], wo_d[:, et])
                    wo_ts.append(wo_t)
                for sg in range(NQ // 4 if 3 in phases else 0):
                    o_sg = p3o.tile([P, HPC, 512], mdt, tag="o_sg")
                    for h in range(HPC):
                        nc.gpsimd.dma_start(
                            o_sg[:, h, :],
                            o_dram[h, :, sg * 512 : (sg + 1) * 512],
                        )
                    for st4 in range(4):
                        for et in range(ET3):
                            ps_y = p3ps.tile([P, 512], f32, tag="ps_y")
                            for w in range(WC3):
                                nc.tensor.matmul(
                                    ps_y[:],
                                    o_sg[:, w, st4 * P : (st4 + 1) * P],
                                    wo_ts[et][:, w, :],
                                    start=(w == 0),
                                    stop=(w == WC3 - 1),
                                )
                            y_sb = p3y.tile([P, 512], f32, tag="y_sb")
                            nc.scalar.copy(y_sb[:], ps_y[:])
                            st = sg * 4 + st4
                            nc.sync.dma_start(
                                y_d[st * P : (st + 1) * P,
                                    et * 512 : (et + 1) * 512],
                                y_sb[:],
                            )

    nc.compile()
    return nc


# ====================== host side: sharding + runner ======================

B, S, D, H = 2, 2048, 4096, 32
HD = D // H
AL = 10
N_CORES = 8
TP = 4  # head groups
HPC = H // TP  # 8 heads per core

_RUNNER = None


def _make_runner(nc, n_cores=N_CORES):
    import jax
    from jax.sharding import Mesh, PartitionSpec
    from jax.experimental.shard_map import shard_map

    from concourse import bass2jax
    from concourse.bass2jax import _bass_exec_p, install_neuronx_cc_hook

    install_neuronx_cc_hook()
    partition_name = nc.partition_id_tensor.name if nc.partition_id_tensor else None

    in_names, out_names, out_avals = [], [], []
    for alloc in nc.m.functions[0].allocations:
        if not isinstance(alloc, mybir.MemoryLocationSet):
            continue
        name = alloc.memorylocations[0].name
        if alloc.kind == "ExternalInput":
            if name != partition_name:
                in_names.append(name)
        elif alloc.kind == "ExternalOutput":
            out_names.append(name)
            out_avals.append(
                jax.core.ShapedArray(
                    tuple(alloc.tensor_shape), mybir.dt.np(alloc.dtype)
                )
            )
    n_params = len(in_names)
    n_outs = len(out_avals)
    all_in_names = list(in_names) + list(out_names)
    if partition_name is not None:
        all_in_names.append(partition_name)

    def _body(*args):
        operands = list(args)
        if partition_name is not None:
            operands.append(bass2jax.partition_id_tensor())
        outs = _bass_exec_p.bind(
            *operands,
            out_avals=tuple(out_avals),
            in_names=tuple(all_in_names),
            out_names=tuple(out_names),
            lowering_input_output_aliases=(),
            sim_require_finite=True,
            sim_require_nnan=True,
            nc=nc,
        )
        return tuple(outs)

    devices = jax.devices()[:n_cores]
    mesh = Mesh(np.asarray(devices), ("core",))
    fn = jax.jit(
        shard_map(
            _body,
            mesh=mesh,
            in_specs=(PartitionSpec("core"),) * (n_params + n_outs),
            out_specs=(PartitionSpec("core"),) * n_outs,
            check_rep=False,
        ),
        keep_unused=True,
    )

    class Runner:
        in_names_ = in_names
        out_names_ = out_names

        def prep(self, in_maps):
            import jax as _jax

            concat_in = [
                np.concatenate(
                    [np.ascontiguousarray(in_maps[c][n]) for c in range(n_cores)],
                    axis=0,
                )
                for n in in_names
            ]
            concat_zero = [
                np.zeros((n_cores * a.shape[0], *a.shape[1:]), a.dtype)
                for a in out_avals
            ]
            shardings = [
                _jax.sharding.NamedSharding(mesh, PartitionSpec("core"))
            ] * (n_params + n_outs)
            return _jax.device_put(concat_in + concat_zero, shardings)

        def run(self, args):
            import jax as _jax

            outs = fn(*args)
            _jax.block_until_ready(outs)
            return [
                {
                    n: np.asarray(outs[i]).reshape(n_cores, *out_avals[i].shape)[c]
                    for i, n in enumerate(out_names)
                }
                for c in range(n_cores)
            ]

        def time_pipelined(self, args, reps=10, warmup=1):
            import time as _time

            import jax as _jax

            for _ in range(warmup):
                _jax.block_until_ready(fn(*args))
            t0 = _time.perf_counter()
            outs = None
            for _ in range(reps):
                outs = fn(*args)
            _jax.block_until_ready(outs)
            return (_time.perf_counter() - t0) / reps

    return Runner()


def _shard_inputs(x, cos, sin, mask, wq, wk, wv, wo, gate, adapter):
    """Build the 8 per-core input maps."""
    import ml_dtypes

    bf = ml_dtypes.bfloat16
    # rope permutation of head-dim columns: even dims first, odd second
    perm = np.concatenate(
        [np.arange(0, HD, 2), np.arange(1, HD, 2)]
    )  # within one head
    col_perm = np.concatenate(
        [h * HD + perm for h in range(H)]
    )  # all heads, head-major
    wq_f = np.asarray(wq, dtype=np.float32)[:, col_perm]
    wk_f = np.asarray(wk, dtype=np.float32)[:, col_perm]
    wv_f = np.asarray(wv, dtype=np.float32)
    wq_p = wq_f.astype(bf)
    wk_p = wk_f.astype(bf)
    wv_b = wv_f.astype(bf)
    wo_b = np.asarray(wo, dtype=np.float32).astype(bf)

    DC = D // P
    HPC_ = HPC
    WC3 = HPC_  # phase-3 contraction chunks (own heads)
    ET3 = D // 512  # full-width out tiles

    def _prearrange_w(w_slice):
        # [D, DH] -> [P, HPC, DC, HD]: contiguous per-partition head tiles
        return np.ascontiguousarray(
            w_slice.reshape(DC, P, HPC_, HD).transpose(1, 2, 0, 3)
        )

    def _prearrange_wo(wo_slice):
        # [DH, D] -> [P, ET3, WC3, 512]
        return np.ascontiguousarray(
            wo_slice.reshape(WC3, P, ET3, 512).transpose(1, 2, 0, 3)
        )

    cosT = np.ascontiguousarray(cos.T, dtype=np.float32)  # [64, S]
    sinT = np.ascontiguousarray(sin.T, dtype=np.float32)

    # 0/1 [k, q] allowed-mask of an aligned 128x128 diagonal block,
    # derived from the mask input (k <= q allowed)
    m = np.asarray(mask, dtype=np.float32)[0, 0]  # [S, S]
    tri = np.ascontiguousarray((m[:P, :P].T == 0)).astype(np.float32)

    gate_v = np.asarray(gate, dtype=np.float32).reshape(H)  # per head
    ad_f = np.asarray(adapter, dtype=np.float32)[0]  # [AL, D]

    xT = [
        np.ascontiguousarray(
            np.asarray(x[b], dtype=np.float32)
            .T.astype(bf)
            .reshape(D // P, P, S)
            .transpose(1, 0, 2)
        )
        for b in range(B)
    ]  # [P, DC, S]

    in_maps = []
    for c in range(N_CORES):
        b = c // TP
        g = c % TP
        hs = g * HPC * HD  # column slice start

        # host-computed adapter projections for this core's heads
        ak = ad_f @ wk_f[:, hs : hs + HPC * HD]  # [AL, HPC*HD], rope basis
        akT = np.ascontiguousarray(
            ak.reshape(AL, HPC_, HD).transpose(2, 1, 0)
        ).astype(bf)  # [P(hd), HPC, AL]
        av = ad_f @ wv_f[:, hs : hs + HPC * HD]  # [AL, HPC*HD]
        av = av.reshape(AL, HPC_, HD) * gate_v[g * HPC : (g + 1) * HPC][None, :, None]
        av = np.ascontiguousarray(av).astype(bf)  # [AL, HPC, P]

        in_maps.append(
            {
                "xT": xT[b],
                "wq": _prearrange_w(wq_p[:, hs : hs + HPC * HD]),
                "wk": _prearrange_w(wk_p[:, hs : hs + HPC * HD]),
                "wv": _prearrange_w(wv_b[:, hs : hs + HPC * HD]),
                "wo": _prearrange_wo(wo_b[hs : hs + HPC * HD, :]),
                "akT": akT,
                "av": av,
                "cosT": cosT,
                "sinT": sinT,
                "tri": tri,
            }
        )
    return in_maps


def get_runner():
    global _RUNNER
    if _RUNNER is None:
        nc = build_nc(Cfg())
        _RUNNER = _make_runner(nc)
    return _RUNNER


def kernel(**inputs) -> np.ndarray:
    x = np.asarray(inputs["x"])
    in_maps = _shard_inputs(
        x,
        inputs["cos"],
        inputs["sin"],
        inputs["mask"],
        inputs["wq"],
        inputs["wk"],
        inputs["wv"],
        inputs["wo"],
        inputs["gate"],
        inputs["adapter"],
    )
    runner = get_runner()
    args = runner.prep(in_maps)
    outs = runner.run(args)
    y = np.zeros((B, S, D), dtype=np.float32)
    for c in range(N_CORES):
        y[c // TP] += outs[c]["y"]
    return y

